# revision 40
# baseline (speedup 1.0000x reference)
"""Trainium2 Bass kernel for nn_EuclideanCodebook (VQ codebook w/ EMA update).

Strategy (data-parallel over tokens, 8 NeuronCores):
  - Each core handles 8192 of the 65536 flattened tokens; the [1024, 128]
    codebook is replicated.
  - Per 128-token tile: PE computes 2*x@e.T into PSUM; the DVE fuses the
    PSUM materialize with the -|e|^2 subtract (tensor_tensor), then does
    max8 + max_index for the exact argmin (fp32, matches jnp.argmin
    bit-for-bit); GPSIMD builds the one-hot row (bf16); PE accumulates
    one_hot.T @ [x|1|0] into a 3-bank packed PSUM region for the segment
    sums (emitted one iteration late so the PE never stalls on the
    PE->DVE->GPSIMD one-hot round trip); an indirect DMA gathers the
    exact quantize rows from the codebook.
  - Segment-sum partials are AllReduce'd across the 8 cores on-device in
    bf16 (counts <= a few hundred stay near-exact; esum enters the EMA
    at weight 0.01, so bf16 noise is ~1e-4 on the outputs), then every
    core computes the EMA tail and writes the codebook outputs; the host
    takes core 0's copy and concatenates the token shards.
"""

import numpy as np

import concourse.bacc as bacc
import concourse.mybir as mybir
from concourse import bass_isa
from concourse.bass import IndirectOffsetOnAxis
from concourse.bass_utils import run_bass_kernel_spmd
from concourse.masks import make_identity
from concourse.tile import TileContext

N_CORES = 8
B, T, D = 64, 1024, 128
K = 1024
N = B * T
N_LOC = N // N_CORES          # 8192 tokens per core
TILES = N_LOC // 128          # 64 tiles of 128 tokens
KC = K // 128                 # 8 codebook chunks of 128 codes
DECAY = 0.99
EPS = 1e-6

FP = mybir.dt.float32
BF = mybir.dt.bfloat16
U32 = mybir.dt.uint32
I32 = mybir.dt.int32

DEBUG_HITS = False


def _build_nc():
    nc = bacc.Bacc("TRN2", target_bir_lowering=False, debug=False,
                   num_devices=N_CORES)

    x_h = nc.dram_tensor("x", [N_LOC, D], FP, kind="ExternalInput")
    e_h = nc.dram_tensor("embed", [K, D], FP, kind="ExternalInput")
    cs_h = nc.dram_tensor("cluster_size", [K], FP, kind="ExternalInput")
    ea_h = nc.dram_tensor("embed_avg", [K, D], FP, kind="ExternalInput")

    q_h = nc.dram_tensor("quantize", [N_LOC, D], FP, kind="ExternalOutput")
    ind_h = nc.dram_tensor("embed_ind", [N_LOC], I32, kind="ExternalOutput")
    csn_h = nc.dram_tensor("cluster_size_new", [K], FP, kind="ExternalOutput")
    ean_h = nc.dram_tensor("embed_avg_new", [K, D], FP, kind="ExternalOutput")
    en_h = nc.dram_tensor("embed_new", [K, D], FP, kind="ExternalOutput")
    hits_h = None
    if DEBUG_HITS:
        hits_h = nc.dram_tensor("hits", [128, TILES], FP, kind="ExternalOutput")

    from contextlib import ExitStack
    with TileContext(nc) as tc, ExitStack() as ctx:
        _kernel(tc, x_h, e_h, cs_h, ea_h, q_h, ind_h, csn_h, ean_h, en_h, ctx,
                hits_h)
    nc.finalize()
    return nc


def _kernel(tc, x_h, e_h, cs_h, ea_h, q_h, ind_h, csn_h, ean_h, en_h,
            ctx, hits_h=None):
    nc = tc.nc
    AL = mybir.AluOpType
    ACT_COPY = mybir.ActivationFunctionType.Copy

    const = ctx.enter_context(tc.tile_pool(name="const", bufs=1))
    sbuf = ctx.enter_context(tc.tile_pool(name="sbuf", bufs=4))
    spool = ctx.enter_context(tc.tile_pool(name="spool", bufs=3))
    psum = ctx.enter_context(tc.tile_pool(name="psum", bufs=2, space="PSUM"))
    espsum = ctx.enter_context(tc.tile_pool(name="espsum", bufs=1, space="PSUM"))
    tpsum = ctx.enter_context(tc.tile_pool(name="tpsum", bufs=1, space="PSUM"))
    dram = ctx.enter_context(tc.tile_pool(name="dram", bufs=1, space="DRAM"))

    # ---------------- setup: constants ----------------
    identity = const.tile([128, 128], FP, tag="identity")
    make_identity(nc, identity[:])

    ones_sta = const.tile([128, 128], FP, tag="ones_sta")
    nc.vector.memset(ones_sta[:], 1.0)

    # embT2 = 2 * embed.T   [D=128, K=1024]
    embT2 = const.tile([128, K], FP, tag="embT2")
    for c in range(KC):
        e_chunk = sbuf.tile([128, 128], FP, tag="e_chunk")
        nc.sync.dma_start(out=e_chunk[:], in_=e_h[c * 128:(c + 1) * 128, :])
        tp = tpsum.tile([128, 128], FP, tag="tp")
        nc.tensor.transpose(out=tp[:], in_=e_chunk[:], identity=identity[:])
        nc.scalar.activation(embT2[:, c * 128:(c + 1) * 128], tp[:],
                             ACT_COPY, scale=2.0)

    # neg_enorm[0, k] = -0.25 * |2 e_k|^2 = -|e_k|^2, as a single row for
    # the per-tile bias matmul (ones-column stationary, contract dim 1).
    esqT = const.tile([128, K], FP, tag="esqT")
    nc.scalar.square(esqT[:], embT2[:])
    enorm_ps = psum.tile([128, K], FP, tag="s_ps")   # shares the s psum tag
    for h in range(2):
        nc.tensor.matmul(enorm_ps[:, h * 512:(h + 1) * 512], ones_sta[:],
                         esqT[:, h * 512:(h + 1) * 512], start=True, stop=True)
    enorm_rep = const.tile([128, K], FP, tag="enorm_rep")
    nc.scalar.activation(enorm_rep[:], enorm_ps[:], ACT_COPY, scale=0.25)

    # persistent accumulators. ES chunks packed 3-per-bank: chunk c lives at
    # es_ps[:, c//3, (c%3)*130 : (c%3)*130+130]  (520B each, 3 banks total)
    es_ps = espsum.tile([128, 3, 512], FP, tag="es_ps")
    idx_all = const.tile([128, TILES], U32, tag="idx_all")

    # moving operands [x | 1 | 0] (bf16): three manually rotated buffers so
    # the constant 1/0 columns are written only once.
    xaugs = []
    for r in range(3):
        xa = const.tile([128, 130], BF, tag=f"xaug{r}")
        nc.vector.memset(xa[:, D:D + 1], 1.0)
        nc.vector.memset(xa[:, D + 1:D + 2], 0.0)
        xaugs.append(xa)
    hits_all = None
    if hits_h is not None:
        hits_all = const.tile([128, TILES], FP, tag="hits_all")

    # Chunks are packed 3 per PSUM bank and start=True clears the whole
    # bank -- so only the first chunk of each bank (c%3==0) clears; the
    # others write into the freshly cleared bank with start=False.
    def _emit_es(item):
        t, oh, xa = item
        for c in range(KC):
            off = (c % 3) * 130
            nc.tensor.matmul(es_ps[:, c // 3, off:off + D + 2],
                             oh[:, c * 128:(c + 1) * 128],
                             xa[:, 0:D + 2],
                             start=(t == 0 and c % 3 == 0),
                             stop=(t == TILES - 1))

    pending = []

    # ---------------- main loop over 64 token tiles ----------------
    for i in range(TILES):
        x_tile = sbuf.tile([128, D], FP, tag="x_tile")
        nc.sync.dma_start(out=x_tile[:], in_=x_h[i * 128:(i + 1) * 128, :])

        # xT for the dist matmul
        xT_ps = tpsum.tile([128, 128], FP, tag="tp")
        nc.tensor.transpose(out=xT_ps[:], in_=x_tile[:], identity=identity[:])
        xT = sbuf.tile([128, 128], FP, tag="xT")
        nc.scalar.copy(xT[:], xT_ps[:])

        # s_psum = 2 x e^T - |e|^2   [tok, 1024]
        # bank h: bias matmul (contract=1, clears the bank) then the dist
        # matmul accumulates on top.
        s_ps = psum.tile([128, K], FP, tag="s_ps")
        for h in range(2):
            nc.tensor.matmul(s_ps[:, h * 512:(h + 1) * 512], xT[:],
                             embT2[:, h * 512:(h + 1) * 512],
                             start=True, stop=True)

        # materialize s = 2xy - |e|^2 (fused bias-subtract) on the DVE
        s_sb = spool.tile([128, K], FP, tag="s_sb")
        nc.vector.tensor_tensor(out=s_sb[:], in0=s_ps[:], in1=enorm_rep[:],
                                op=AL.subtract)

        # exact argmax of s == argmin of dist
        top8 = sbuf.tile([128, 8], FP, tag="top8")
        nc.vector.max(out=top8[:], in_=s_sb[:])
        idx8 = sbuf.tile([128, 8], U32, tag="idx8")
        nc.vector.max_index(out=idx8[:], in_max=top8[:], in_values=s_sb[:])
        nc.gpsimd.tensor_copy(idx_all[:, i:i + 1], idx8[:, 0:1])

        # one-hot row (bf16) for the segment-sum matmul
        onehot = spool.tile([128, K], BF, tag="onehot")
        nc.gpsimd.tensor_scalar(out=onehot[:], in0=s_sb[:],
                                scalar1=top8[:, 0:1], scalar2=None,
                                op0=AL.is_equal)

        if hits_all is not None:
            nc.vector.tensor_reduce(out=hits_all[:, i:i + 1], in_=onehot[:],
                                    axis=mybir.AxisListType.X, op=AL.add)

        # moving operand [x | 1 | 0] in bf16 (manually rotated const buffer)
        xaug = xaugs[i % 3]
        nc.gpsimd.tensor_copy(xaug[:, 0:D], x_tile[:])

        # segment sums: es[c*128+j, :] += one_hot.T @ [x | 1 | 0].
        # Emitted one iteration late so the PE never stalls on the
        # PE->ACT->DVE->GPSIMD one-hot round trip of the current tile.
        pending.append((i, onehot, xaug))
        if i >= 1:
            _emit_es(pending.pop(0))

        # quantize rows: gather embed[idx]
        q_tile = sbuf.tile([128, D], FP, tag="q_tile")
        nc.gpsimd.indirect_dma_start(
            out=q_tile[:], out_offset=None, in_=e_h[:, :],
            in_offset=IndirectOffsetOnAxis(ap=idx8[:, 0:1], axis=0))
        # ACT-ring HWDGE: keeps the SP ring free for the x loads (same-ring
        # FIFO ordering would chain x-load(i+1) behind the whole tile-i path)
        nc.scalar.dma_start(out=q_h[i * 128:(i + 1) * 128, :], in_=q_tile[:])

    while pending:
        _emit_es(pending.pop(0))

    if hits_all is not None:
        nc.sync.dma_start(out=hits_h[:, :], in_=hits_all[:])

    # ---------------- embed_ind output ----------------
    # idx_all[:, ::8] is [128 tok-in-tile, 64 tiles]; transpose -> [tile, tok]
    idx_f = sbuf.tile([128, TILES], FP, tag="idx_f")
    nc.vector.tensor_copy(idx_f[:], idx_all[:])
    idxT_ps = tpsum.tile([128, 128], FP, tag="tp")
    nc.tensor.transpose(out=idxT_ps[0:TILES, 0:128], in_=idx_f[:],
                        identity=identity[:])
    idx_out = sbuf.tile([TILES, 128], I32, tag="idx_out")
    nc.vector.tensor_copy(idx_out[:], idxT_ps[0:TILES, 0:128])
    nc.sync.dma_start(out=ind_h.ap().rearrange("(t p) -> t p", p=128),
                      in_=idx_out[:])

    # ---------------- all-reduce the segment sums ----------------
    # es_ps chunks live at [:, c//3, (c%3)*130 +: 130]; view as 9 chunks of
    # 130 (the 9th is zero filler from the bank clear).
    es_view = es_ps[:, :, 0:390].rearrange("p a (b r) -> p a b r", r=130)
    es_sb = const.tile([128, 9, 130], BF, tag="es_sb")
    nc.scalar.copy(es_sb[:].rearrange("p (a b) r -> p a b r", b=3), es_view)
    cc_in = dram.tile([128, 9, 130], BF, tag="cc_in")
    cc_out = dram.tile([128, 9, 130], BF, tag="cc_out")
    nc.sync.dma_start(out=cc_in[:], in_=es_sb[:])
    nc.gpsimd.collective_compute(
        "AllReduce", AL.add,
        replica_groups=[list(range(N_CORES))],
        ins=[cc_in[:].opt()], outs=[cc_out[:].opt()])
    esr = const.tile([128, 9, 130], BF, tag="esr")
    nc.sync.dma_start(out=esr[:], in_=cc_out[:])

    # ---------------- EMA tail ----------------
    # layout: code k = c*128 + p  ->  [p, c]
    cs_sb = const.tile([128, KC], FP, tag="cs_sb")
    nc.sync.dma_start(out=cs_sb[:],
                      in_=cs_h.ap().rearrange("(c p) -> p c", p=128))
    ea_sb = const.tile([128, KC, D], FP, tag="ea_sb")
    nc.sync.dma_start(out=ea_sb[:],
                      in_=ea_h.ap().rearrange("(c p) d -> p c d", p=128))

    counts = esr[:, 0:KC, D]                   # [128, KC]
    esum = esr[:, 0:KC, 0:D]                   # [128, KC, D]

    # cluster_size_new = cs*decay + counts*(1-decay)
    csn = const.tile([128, KC], FP, tag="csn")
    nc.vector.tensor_scalar_mul(csn[:], cs_sb[:], DECAY)
    tmp_c = sbuf.tile([128, KC], FP, tag="tmp_c")
    nc.vector.tensor_scalar_mul(tmp_c[:], counts, 1.0 - DECAY)
    nc.vector.tensor_add(csn[:], csn[:], tmp_c[:])
    nc.sync.dma_start(out=csn_h.ap().rearrange("(c p) -> p c", p=128),
                      in_=csn[:])

    # embed_avg_new = ea*decay + esum*(1-decay)
    ean = const.tile([128, KC, D], FP, tag="ean")
    nc.vector.tensor_scalar_mul(ean[:], ea_sb[:], DECAY)
    esum_s = const.tile([128, KC, D], FP, tag="esum_s")
    nc.vector.tensor_scalar_mul(esum_s[:], esum, 1.0 - DECAY)
    nc.vector.tensor_add(ean[:], ean[:], esum_s[:])
    nc.sync.dma_start(out=ean_h.ap().rearrange("(c p) d -> p c d", p=128),
                      in_=ean[:])

    # total = sum(csn); smoothed = (csn + eps) / (total + eps*K) * total
    part = sbuf.tile([128, 1], FP, tag="part")
    nc.vector.tensor_reduce(out=part[:], in_=csn[:], axis=mybir.AxisListType.X,
                            op=AL.add)
    total = sbuf.tile([128, 1], FP, tag="total")
    nc.gpsimd.partition_all_reduce(total[:], part[:], 128,
                                   bass_isa.ReduceOp.add)
    denom = sbuf.tile([128, 1], FP, tag="denom")
    nc.vector.tensor_scalar_add(denom[:], total[:], EPS * K)
    rden = sbuf.tile([128, 1], FP, tag="rden")
    nc.vector.reciprocal(rden[:], denom[:])
    fac = sbuf.tile([128, 1], FP, tag="fac")          # total / (total+eps*K)
    nc.vector.tensor_mul(fac[:], total[:], rden[:])
    smoothed = sbuf.tile([128, KC], FP, tag="smoothed")
    nc.vector.tensor_scalar(out=smoothed[:], in0=csn[:], scalar1=EPS,
                            scalar2=fac[:, 0:1], op0=AL.add, op1=AL.mult)
    rsm = sbuf.tile([128, KC], FP, tag="rsm")
    nc.vector.reciprocal(rsm[:], smoothed[:])

    # embed_new = embed_avg_new / smoothed[:, None]
    en = const.tile([128, KC, D], FP, tag="en")
    for c in range(KC):
        nc.vector.tensor_scalar_mul(en[:, c, :], ean[:, c, :], rsm[:, c:c + 1])
    nc.sync.dma_start(out=en_h.ap().rearrange("(c p) d -> p c d", p=128),
                      in_=en[:])


_NC_CACHE = None


def _get_nc():
    global _NC_CACHE
    if _NC_CACHE is None:
        _NC_CACHE = _build_nc()
    return _NC_CACHE


def kernel(x, embed, cluster_size, embed_avg):
    x = np.ascontiguousarray(np.asarray(x, dtype=np.float32))
    embed = np.ascontiguousarray(np.asarray(embed, dtype=np.float32))
    cluster_size = np.ascontiguousarray(np.asarray(cluster_size, dtype=np.float32))
    embed_avg = np.ascontiguousarray(np.asarray(embed_avg, dtype=np.float32))

    shape = x.shape
    xf = x.reshape(-1, shape[-1])
    nc = _get_nc()

    in_maps = []
    for c in range(N_CORES):
        in_maps.append({
            "x": xf[c * N_LOC:(c + 1) * N_LOC],
            "embed": embed,
            "cluster_size": cluster_size,
            "embed_avg": embed_avg,
        })
    res = run_bass_kernel_spmd(nc, in_maps, core_ids=list(range(N_CORES)))
    outs = res.results

    quantize = np.concatenate([outs[c]["quantize"] for c in range(N_CORES)],
                              axis=0).reshape(shape)
    embed_ind = np.concatenate([outs[c]["embed_ind"] for c in range(N_CORES)],
                               axis=0).reshape(shape[:-1]).astype(np.int32)
    cluster_size_new = outs[0]["cluster_size_new"]
    embed_avg_new = outs[0]["embed_avg_new"]
    embed_new = outs[0]["embed_new"]
    return quantize, embed_ind, cluster_size_new, embed_avg_new, embed_new


# revision 41
# speedup vs baseline: 1.0047x; 1.0047x over previous
"""Trainium2 Bass kernel for nn_EuclideanCodebook (VQ codebook w/ EMA update).

Strategy (data-parallel over tokens, 8 NeuronCores):
  - Each core handles 8192 of the 65536 flattened tokens; the [1024, 128]
    codebook is replicated.
  - Per 128-token tile: PE computes 2*x@e.T into PSUM; the DVE fuses the
    PSUM materialize with the -|e|^2 subtract (tensor_tensor), then does
    max8 + max_index for the exact argmin (fp32, matches jnp.argmin
    bit-for-bit); GPSIMD builds the one-hot row (bf16); PE accumulates
    one_hot.T @ [x|1|0] into a 3-bank packed PSUM region for the segment
    sums (emitted one iteration late so the PE never stalls on the
    PE->DVE->GPSIMD one-hot round trip); an indirect DMA gathers the
    exact quantize rows from the codebook.
  - Segment-sum partials are AllReduce'd across the 8 cores on-device in
    bf16 (counts <= a few hundred stay near-exact; esum enters the EMA
    at weight 0.01, so bf16 noise is ~1e-4 on the outputs), then every
    core computes the EMA tail and writes the codebook outputs; the host
    takes core 0's copy and concatenates the token shards.
"""

import numpy as np

import concourse.bacc as bacc
import concourse.mybir as mybir
from concourse import bass_isa
from concourse.bass import IndirectOffsetOnAxis
from concourse.bass_utils import run_bass_kernel_spmd
from concourse.masks import make_identity
from concourse.tile import TileContext

N_CORES = 8
B, T, D = 64, 1024, 128
K = 1024
N = B * T
N_LOC = N // N_CORES          # 8192 tokens per core
TILES = N_LOC // 128          # 64 tiles of 128 tokens
KC = K // 128                 # 8 codebook chunks of 128 codes
DECAY = 0.99
EPS = 1e-6

FP = mybir.dt.float32
BF = mybir.dt.bfloat16
U32 = mybir.dt.uint32
I32 = mybir.dt.int32

DEBUG_HITS = False


def _build_nc():
    nc = bacc.Bacc("TRN2", target_bir_lowering=False, debug=False,
                   num_devices=N_CORES)

    x_h = nc.dram_tensor("x", [N_LOC, D], FP, kind="ExternalInput")
    e_h = nc.dram_tensor("embed", [K, D], FP, kind="ExternalInput")
    cs_h = nc.dram_tensor("cluster_size", [K], FP, kind="ExternalInput")
    ea_h = nc.dram_tensor("embed_avg", [K, D], FP, kind="ExternalInput")

    q_h = nc.dram_tensor("quantize", [N_LOC, D], FP, kind="ExternalOutput")
    ind_h = nc.dram_tensor("embed_ind", [N_LOC], I32, kind="ExternalOutput")
    csn_h = nc.dram_tensor("cluster_size_new", [K], FP, kind="ExternalOutput")
    ean_h = nc.dram_tensor("embed_avg_new", [K, D], FP, kind="ExternalOutput")
    en_h = nc.dram_tensor("embed_new", [K, D], FP, kind="ExternalOutput")
    hits_h = None
    if DEBUG_HITS:
        hits_h = nc.dram_tensor("hits", [128, TILES], FP, kind="ExternalOutput")

    from contextlib import ExitStack
    with TileContext(nc) as tc, ExitStack() as ctx:
        _kernel(tc, x_h, e_h, cs_h, ea_h, q_h, ind_h, csn_h, ean_h, en_h, ctx,
                hits_h)
    nc.finalize()
    return nc


def _kernel(tc, x_h, e_h, cs_h, ea_h, q_h, ind_h, csn_h, ean_h, en_h,
            ctx, hits_h=None):
    nc = tc.nc
    AL = mybir.AluOpType
    ACT_COPY = mybir.ActivationFunctionType.Copy

    const = ctx.enter_context(tc.tile_pool(name="const", bufs=1))
    sbuf = ctx.enter_context(tc.tile_pool(name="sbuf", bufs=4))
    spool = ctx.enter_context(tc.tile_pool(name="spool", bufs=3))
    psum = ctx.enter_context(tc.tile_pool(name="psum", bufs=2, space="PSUM"))
    espsum = ctx.enter_context(tc.tile_pool(name="espsum", bufs=1, space="PSUM"))
    tpsum = ctx.enter_context(tc.tile_pool(name="tpsum", bufs=1, space="PSUM"))
    dram = ctx.enter_context(tc.tile_pool(name="dram", bufs=1, space="DRAM"))

    # ---------------- setup: constants ----------------
    identity = const.tile([128, 128], FP, tag="identity")
    make_identity(nc, identity[:])

    ones_sta = const.tile([128, 128], FP, tag="ones_sta")
    nc.vector.memset(ones_sta[:], 1.0)

    # embT2 = 2 * embed.T   [D=128, K=1024]
    embT2 = const.tile([128, K], FP, tag="embT2")
    for c in range(KC):
        e_chunk = sbuf.tile([128, 128], FP, tag="e_chunk")
        nc.sync.dma_start(out=e_chunk[:], in_=e_h[c * 128:(c + 1) * 128, :])
        tp = tpsum.tile([128, 128], FP, tag="tp")
        nc.tensor.transpose(out=tp[:], in_=e_chunk[:], identity=identity[:])
        nc.scalar.activation(embT2[:, c * 128:(c + 1) * 128], tp[:],
                             ACT_COPY, scale=2.0)

    # neg_enorm[0, k] = -0.25 * |2 e_k|^2 = -|e_k|^2, as a single row for
    # the per-tile bias matmul (ones-column stationary, contract dim 1).
    esqT = const.tile([128, K], FP, tag="esqT")
    nc.scalar.square(esqT[:], embT2[:])
    enorm_ps = psum.tile([128, K], FP, tag="s_ps")   # shares the s psum tag
    for h in range(2):
        nc.tensor.matmul(enorm_ps[:, h * 512:(h + 1) * 512], ones_sta[:],
                         esqT[:, h * 512:(h + 1) * 512], start=True, stop=True)
    enorm_rep = const.tile([128, K], FP, tag="enorm_rep")
    nc.scalar.activation(enorm_rep[:], enorm_ps[:], ACT_COPY, scale=0.25)

    # persistent accumulators. ES chunks packed 3-per-bank: chunk c lives at
    # es_ps[:, c//3, (c%3)*130 : (c%3)*130+130]  (520B each, 3 banks total)
    es_ps = espsum.tile([128, 3, 512], FP, tag="es_ps")
    idx_all = const.tile([128, TILES], U32, tag="idx_all")

    # moving operands [x | 1 | 0] (bf16): three manually rotated buffers so
    # the constant 1/0 columns are written only once.
    xaugs = []
    for r in range(3):
        xa = const.tile([128, 130], BF, tag=f"xaug{r}")
        nc.vector.memset(xa[:, D:D + 1], 1.0)
        nc.vector.memset(xa[:, D + 1:D + 2], 0.0)
        xaugs.append(xa)
    hits_all = None
    if hits_h is not None:
        hits_all = const.tile([128, TILES], FP, tag="hits_all")

    # Chunks are packed 3 per PSUM bank and start=True clears the whole
    # bank -- so only the first chunk of each bank (c%3==0) clears; the
    # others write into the freshly cleared bank with start=False.
    def _emit_es(item):
        t, oh, xa = item
        for c in range(KC):
            off = (c % 3) * 130
            nc.tensor.matmul(es_ps[:, c // 3, off:off + D + 2],
                             oh[:, c * 128:(c + 1) * 128],
                             xa[:, 0:D + 2],
                             start=(t == 0 and c % 3 == 0),
                             stop=(t == TILES - 1))

    pending = []

    # ---------------- main loop over 64 token tiles ----------------
    for i in range(TILES):
        x_tile = sbuf.tile([128, D], FP, tag="x_tile")
        nc.sync.dma_start(out=x_tile[:], in_=x_h[i * 128:(i + 1) * 128, :])

        # xT for the dist matmul
        xT_ps = tpsum.tile([128, 128], FP, tag="tp")
        nc.tensor.transpose(out=xT_ps[:], in_=x_tile[:], identity=identity[:])
        xT = sbuf.tile([128, 128], FP, tag="xT")
        nc.scalar.copy(xT[:], xT_ps[:])

        # s_psum = 2 x e^T - |e|^2   [tok, 1024]
        # bank h: bias matmul (contract=1, clears the bank) then the dist
        # matmul accumulates on top.
        s_ps = psum.tile([128, K], FP, tag="s_ps")
        for h in range(2):
            nc.tensor.matmul(s_ps[:, h * 512:(h + 1) * 512], xT[:],
                             embT2[:, h * 512:(h + 1) * 512],
                             start=True, stop=True)

        # materialize s = 2xy - |e|^2 (fused bias-subtract) on the DVE
        s_sb = spool.tile([128, K], FP, tag="s_sb")
        nc.vector.tensor_tensor(out=s_sb[:], in0=s_ps[:], in1=enorm_rep[:],
                                op=AL.subtract)

        # exact argmax of s == argmin of dist
        top8 = sbuf.tile([128, 8], FP, tag="top8")
        nc.vector.max(out=top8[:], in_=s_sb[:])
        idx8 = sbuf.tile([128, 8], U32, tag="idx8")
        nc.vector.max_index(out=idx8[:], in_max=top8[:], in_values=s_sb[:])
        nc.gpsimd.tensor_copy(idx_all[:, i:i + 1], idx8[:, 0:1])

        # one-hot row (bf16) for the segment-sum matmul
        onehot = spool.tile([128, K], BF, tag="onehot")
        nc.gpsimd.tensor_scalar(out=onehot[:], in0=s_sb[:],
                                scalar1=top8[:, 0:1], scalar2=None,
                                op0=AL.is_equal)

        if hits_all is not None:
            nc.vector.tensor_reduce(out=hits_all[:, i:i + 1], in_=onehot[:],
                                    axis=mybir.AxisListType.X, op=AL.add)

        # moving operand [x | 1 | 0] in bf16 (manually rotated const buffer)
        xaug = xaugs[i % 3]
        nc.gpsimd.tensor_copy(xaug[:, 0:D], x_tile[:])

        # segment sums: es[c*128+j, :] += one_hot.T @ [x | 1 | 0].
        # Emitted one iteration late so the PE never stalls on the
        # PE->ACT->DVE->GPSIMD one-hot round trip of the current tile.
        pending.append((i, onehot, xaug))
        if i >= 1:
            _emit_es(pending.pop(0))

        # quantize rows: gather embed[idx]
        q_tile = sbuf.tile([128, D], FP, tag="q_tile")
        nc.gpsimd.indirect_dma_start(
            out=q_tile[:], out_offset=None, in_=e_h[:, :],
            in_offset=IndirectOffsetOnAxis(ap=idx8[:, 0:1], axis=0))
        # ACT-ring HWDGE: keeps the SP ring free for the x loads (same-ring
        # FIFO ordering would chain x-load(i+1) behind the whole tile-i path)
        nc.scalar.dma_start(out=q_h[i * 128:(i + 1) * 128, :], in_=q_tile[:])

    while pending:
        _emit_es(pending.pop(0))

    if hits_all is not None:
        nc.sync.dma_start(out=hits_h[:, :], in_=hits_all[:])

    # ---------------- embed_ind output ----------------
    # idx_all[:, ::8] is [128 tok-in-tile, 64 tiles]; transpose -> [tile, tok]
    idx_f = sbuf.tile([128, TILES], FP, tag="idx_f")
    nc.vector.tensor_copy(idx_f[:], idx_all[:])
    idxT_ps = tpsum.tile([128, 128], FP, tag="tp")
    nc.tensor.transpose(out=idxT_ps[0:TILES, 0:128], in_=idx_f[:],
                        identity=identity[:])
    idx_out = sbuf.tile([TILES, 128], I32, tag="idx_out")
    nc.vector.tensor_copy(idx_out[:], idxT_ps[0:TILES, 0:128])
    nc.sync.dma_start(out=ind_h.ap().rearrange("(t p) -> t p", p=128),
                      in_=idx_out[:])

    # ---------------- all-reduce the segment sums ----------------
    # es_ps chunks live at [:, c//3, (c%3)*130 +: 130]; view as 9 chunks of
    # 130 (the 9th is zero filler from the bank clear).
    es_sb = const.tile([128, 8, 130], BF, tag="es_sb")
    nc.scalar.copy(
        es_sb[:, 0:6, :].rearrange("p (a b) r -> p a b r", b=3),
        es_ps[:, 0:2, 0:390].rearrange("p a (b r) -> p a b r", r=130))
    nc.scalar.copy(
        es_sb[:, 6:8, :].rearrange("p (a b) r -> p a b r", b=2),
        es_ps[:, 2:3, 0:260].rearrange("p a (b r) -> p a b r", r=130))
    cc_in = dram.tile([128, 8, 130], BF, tag="cc_in")
    cc_out = dram.tile([128, 8, 130], BF, tag="cc_out")
    nc.sync.dma_start(out=cc_in[:], in_=es_sb[:])
    nc.gpsimd.collective_compute(
        "AllReduce", AL.add,
        replica_groups=[list(range(N_CORES))],
        ins=[cc_in[:].opt()], outs=[cc_out[:].opt()])
    esr = const.tile([128, 8, 130], BF, tag="esr")
    nc.sync.dma_start(out=esr[:], in_=cc_out[:])

    # ---------------- EMA tail ----------------
    # layout: code k = c*128 + p  ->  [p, c]
    cs_sb = const.tile([128, KC], FP, tag="cs_sb")
    nc.sync.dma_start(out=cs_sb[:],
                      in_=cs_h.ap().rearrange("(c p) -> p c", p=128))
    ea_sb = const.tile([128, KC, D], FP, tag="ea_sb")
    nc.sync.dma_start(out=ea_sb[:],
                      in_=ea_h.ap().rearrange("(c p) d -> p c d", p=128))

    counts = esr[:, 0:KC, D]                   # [128, KC]
    esum = esr[:, 0:KC, 0:D]                   # [128, KC, D]

    # cluster_size_new = cs*decay + counts*(1-decay)
    csn = const.tile([128, KC], FP, tag="csn")
    nc.vector.tensor_scalar_mul(csn[:], cs_sb[:], DECAY)
    tmp_c = sbuf.tile([128, KC], FP, tag="tmp_c")
    nc.vector.tensor_scalar_mul(tmp_c[:], counts, 1.0 - DECAY)
    nc.vector.tensor_add(csn[:], csn[:], tmp_c[:])
    nc.sync.dma_start(out=csn_h.ap().rearrange("(c p) -> p c", p=128),
                      in_=csn[:])

    # embed_avg_new = ea*decay + esum*(1-decay)
    ean = const.tile([128, KC, D], FP, tag="ean")
    nc.vector.tensor_scalar_mul(ean[:], ea_sb[:], DECAY)
    esum_s = const.tile([128, KC, D], FP, tag="esum_s")
    nc.vector.tensor_scalar_mul(esum_s[:], esum, 1.0 - DECAY)
    nc.vector.tensor_add(ean[:], ean[:], esum_s[:])
    nc.sync.dma_start(out=ean_h.ap().rearrange("(c p) d -> p c d", p=128),
                      in_=ean[:])

    # total = sum(csn); smoothed = (csn + eps) / (total + eps*K) * total
    part = sbuf.tile([128, 1], FP, tag="part")
    nc.vector.tensor_reduce(out=part[:], in_=csn[:], axis=mybir.AxisListType.X,
                            op=AL.add)
    total = sbuf.tile([128, 1], FP, tag="total")
    nc.gpsimd.partition_all_reduce(total[:], part[:], 128,
                                   bass_isa.ReduceOp.add)
    denom = sbuf.tile([128, 1], FP, tag="denom")
    nc.vector.tensor_scalar_add(denom[:], total[:], EPS * K)
    rden = sbuf.tile([128, 1], FP, tag="rden")
    nc.vector.reciprocal(rden[:], denom[:])
    fac = sbuf.tile([128, 1], FP, tag="fac")          # total / (total+eps*K)
    nc.vector.tensor_mul(fac[:], total[:], rden[:])
    smoothed = sbuf.tile([128, KC], FP, tag="smoothed")
    nc.vector.tensor_scalar(out=smoothed[:], in0=csn[:], scalar1=EPS,
                            scalar2=fac[:, 0:1], op0=AL.add, op1=AL.mult)
    rsm = sbuf.tile([128, KC], FP, tag="rsm")
    nc.vector.reciprocal(rsm[:], smoothed[:])

    # embed_new = embed_avg_new / smoothed[:, None]
    en = const.tile([128, KC, D], FP, tag="en")
    for c in range(KC):
        nc.vector.tensor_scalar_mul(en[:, c, :], ean[:, c, :], rsm[:, c:c + 1])
    nc.sync.dma_start(out=en_h.ap().rearrange("(c p) d -> p c d", p=128),
                      in_=en[:])


_NC_CACHE = None


def _get_nc():
    global _NC_CACHE
    if _NC_CACHE is None:
        _NC_CACHE = _build_nc()
    return _NC_CACHE


def kernel(x, embed, cluster_size, embed_avg):
    x = np.ascontiguousarray(np.asarray(x, dtype=np.float32))
    embed = np.ascontiguousarray(np.asarray(embed, dtype=np.float32))
    cluster_size = np.ascontiguousarray(np.asarray(cluster_size, dtype=np.float32))
    embed_avg = np.ascontiguousarray(np.asarray(embed_avg, dtype=np.float32))

    shape = x.shape
    xf = x.reshape(-1, shape[-1])
    nc = _get_nc()

    in_maps = []
    for c in range(N_CORES):
        in_maps.append({
            "x": xf[c * N_LOC:(c + 1) * N_LOC],
            "embed": embed,
            "cluster_size": cluster_size,
            "embed_avg": embed_avg,
        })
    res = run_bass_kernel_spmd(nc, in_maps, core_ids=list(range(N_CORES)))
    outs = res.results

    quantize = np.concatenate([outs[c]["quantize"] for c in range(N_CORES)],
                              axis=0).reshape(shape)
    embed_ind = np.concatenate([outs[c]["embed_ind"] for c in range(N_CORES)],
                               axis=0).reshape(shape[:-1]).astype(np.int32)
    cluster_size_new = outs[0]["cluster_size_new"]
    embed_avg_new = outs[0]["embed_avg_new"]
    embed_new = outs[0]["embed_new"]
    return quantize, embed_ind, cluster_size_new, embed_avg_new, embed_new
